# revision 14
# baseline (speedup 1.0000x reference)
"""Trainium2 Bass kernel for nn_Atom3D (5-layer GCN + global pool + MLP head).

Distribution: graph-aligned node sharding across 8 NeuronCores (8 graphs/core).
Per layer: edge messages move via dma_gather (send-side row gather, per dest
chunk), two pipelined AllToAll collectives (half A / half B), and
dma_scatter_add (CCE accumulate) onto the local node table; the dense matmul +
folded-BN epilogue runs locally.

Layout: a2a buffers are partition-major blocks [128, TQ, w] per (dest) chunk so
bounce DMAs are big contiguous transfers.  The HW CCE scatter-add loses
concurrent adds to the same row, so scatter waves are t-slot ranges across all
8 chunks and the host assigns same-dst edges to distinct waves.

Math factoring (validated vs reference to ~4e-6):
  - GCN symmetric norm is separable: ne = dis[src]*dis[dst].  Tables store
    h~ = dis * h, messages are raw rows, and the dst-side dis multiplies the
    matmul *output* rows (diag row-scale commutes through z @ W).
  - BN (eval mode) scale s is folded into W columns (s > 0); bias b and the
    BN shift t enter PSUM as rank-1 matmuls (1/dis and kappa rows); the t of
    layers 1-3 is pushed through the linear aggregation into layer l+1 using
    kappa = A' @ dis (host-precomputed).
"""

from dataclasses import dataclass

import numpy as np
import ml_dtypes

BF16NP = ml_dtypes.bfloat16
BN_EPS = 1e-5
DIMS = [(128, 128), (128, 256), (256, 512), (512, 512), (512, 512)]


@dataclass(frozen=True)
class Cfg:
    n: int = 80000
    g: int = 64
    c: int = 8
    np_cap: int = 10240      # per-core node rows (multiple of 128)
    pair_cap: int = 3072     # per (src-core, dst-core) edge slots (mult of 256)
    nw: int = 12             # scatter waves (>= max in-degree + slack)
    write_tables: bool = False

    @property
    def npt(self) -> int:
        return self.np_cap + 128  # + trash tile (dummy-edge scatter target)

    @property
    def sendcap(self) -> int:
        return self.c * self.pair_cap

    @property
    def tq(self) -> int:
        assert self.pair_cap % 128 == 0
        return self.pair_cap // 128  # t-slots per chunk

    @property
    def wave_tr(self) -> list[tuple[int, int]]:
        """Scatter-wave t-ranges [(t0, t1)) covering 0..TQ, nw of them."""
        tq, nw = self.tq, self.nw
        base, extra = divmod(tq, nw)
        out = []
        t = 0
        for i in range(nw):
            sz = base + (1 if i < extra else 0)
            out.append((t, t + sz))
            t += sz
        assert t == tq and all(b > a for a, b in out)
        return out


FULL_CFG = Cfg()


def _wrap_idx(ids: np.ndarray) -> np.ndarray:
    """int16 index array -> [128, len/16] wrapped layout (16-partition pattern
    replicated 8x for the Q7 cores): idx i lives at [i % 16 (+16k), i // 16]."""
    assert len(ids) % 16 == 0
    return np.ascontiguousarray(np.tile(ids.reshape(-1, 16).T, (8, 1)).astype(np.int16))


def preprocess(x, ei_intra, ei_inter, batch, params, cfg: Cfg):
    """All-numpy host prep. Returns in_maps (one dict per core)."""
    N, G, C = cfg.n, cfg.g, cfg.c
    gpc = G // C
    x = np.asarray(x, np.float64)
    batch = np.asarray(batch)
    src = np.concatenate([np.asarray(ei_intra)[0], np.asarray(ei_inter)[0]])
    dst = np.concatenate([np.asarray(ei_intra)[1], np.asarray(ei_inter)[1]])

    deg = np.bincount(dst, minlength=N).astype(np.float64) + 1.0
    dis = 1.0 / np.sqrt(deg)
    kappa = np.bincount(dst, weights=dis[src], minlength=N) + dis

    gcounts = np.bincount(batch, minlength=G)
    gstart = np.concatenate([[0], np.cumsum(gcounts)]).astype(np.int64)
    core_start = gstart[0 : G : gpc]
    core_end = gstart[gpc :: gpc]
    ncnt = core_end - core_start
    assert ncnt.max() <= cfg.np_cap, (ncnt.max(), cfg.np_cap)
    core_of = np.zeros(N, np.int64)
    for c in range(C):
        core_of[core_start[c] : core_end[c]] = c

    p = {k: np.asarray(v, np.float64) for k, v in params.items()}
    s, t, Wf, bt, tau = {}, {}, {}, {}, {}
    for l in range(1, 6):
        s[l] = p[f"g{l}"] / np.sqrt(p[f"v{l}"] + BN_EPS)
        assert (s[l] > 0).all(), "BN scale sign flip not supported"
        t[l] = p[f"beta{l}"] - p[f"m{l}"] * s[l]
        Wf[l] = p[f"W{l}"] * s[l][None, :]
    for l in range(1, 6):
        bt[l] = s[l] * p[f"b{l}"]
        if l in (4, 5):
            bt[l] = bt[l] + t[l]
        tau[l] = t[l - 1] @ Wf[l] if l in (2, 3, 4) else np.zeros(DIMS[l - 1][1])

    score = core_of[src]
    dcore = core_of[dst]
    pair_g = [[None] * C for _ in range(C)]  # [o][c] sender-local src rows
    pair_s = [[None] * C for _ in range(C)]  # [o][c] receiver-local dst rows
    for o in range(C):
        mo = score == o
        for c in range(C):
            e = np.nonzero(mo & (dcore == c))[0]
            assert len(e) <= cfg.pair_cap, (o, c, len(e), cfg.pair_cap)
            pair_g[o][c] = (src[e] - core_start[o]).astype(np.int16)
            pair_s[o][c] = (dst[e] - core_start[c]).astype(np.int16)

    NPT, NT = cfg.npt, cfg.np_cap // 128
    TRASH = cfg.np_cap
    NW, TQ = cfg.nw, cfg.tq
    tr = cfg.wave_tr
    wave_of_t = np.zeros(TQ, np.int64)
    for wv, (a, b) in enumerate(tr):
        wave_of_t[a:b] = wv
    slots_per_wave = [(b - a) * 128 for a, b in tr]

    # Assign each receiver's edges to (chunk o, t-slot, p): same-dst edges in
    # distinct waves (t-ranges). Slot (o, t, p) <-> chunk-local pos t*128+p.
    recv_dst = []   # [c] -> [C, TQ, 128] receiver-local dst (TRASH = dummy)
    send_srcm = []  # [c] -> [C, TQ, 128] sender(o)-local src rows (0 = dummy)
    from collections import defaultdict

    for c in range(C):
        by_dst = defaultdict(list)
        for o in range(C):
            for srcl, dstl in zip(pair_g[o][c], pair_s[o][c]):
                by_dst[int(dstl)].append((o, int(srcl)))
        fill = np.zeros((C, NW), np.int64)  # slots used per (chunk, wave)
        sdst = np.full((C, TQ, 128), TRASH, np.int16)
        ssrc = np.zeros((C, TQ, 128), np.int16)
        for d, elist in sorted(by_dst.items(), key=lambda kv: -len(kv[1])):
            used = set()
            for o, srcl in elist:
                cands = [
                    w for w in range(NW)
                    if w not in used and fill[o][w] < slots_per_wave[w]
                ]
                assert cands, f"wave assignment infeasible (core {c}, dst {d})"
                w = min(cands, key=lambda ww: fill[o][ww] / slots_per_wave[ww])
                k = fill[o][w]
                fill[o][w] += 1
                t0 = tr[w][0] + k // 128
                sdst[o, t0, k % 128] = d
                ssrc[o, t0, k % 128] = srcl
                used.add(w)
        recv_dst.append(sdst)
        send_srcm.append(ssrc)

    def shared_tensors():
        m = {}
        for l in range(1, 6):
            m[f"w{l}"] = np.ascontiguousarray(Wf[l].astype(BF16NP))
            m[f"r1b{l}"] = np.ascontiguousarray(np.stack([bt[l], tau[l]]).astype(BF16NP))
        m["fw1"] = np.ascontiguousarray(p["fcW1"].astype(np.float32))
        m["fb1"] = np.ascontiguousarray(p["fcb1"].astype(np.float32)[None, :])
        m["fw2"] = np.ascontiguousarray(p["fcW2"].astype(np.float32))
        m["fb2"] = np.ascontiguousarray(p["fcb2"].astype(np.float32)[None, :])
        return m

    shared = shared_tensors()
    in_maps = []
    for c in range(C):
        n0, n1 = core_start[c], core_end[c]
        cnt = n1 - n0
        m = dict(shared)

        xt = np.zeros((NPT, 128), np.float64)
        xt[:cnt] = dis[n0:n1, None] * x[n0:n1]
        m["xt"] = np.ascontiguousarray(xt.astype(BF16NP))

        disl = np.ones(NPT, np.float64)
        disl[:cnt] = dis[n0:n1]
        m["disr"] = np.ascontiguousarray(
            disl[: cfg.np_cap].reshape(NT, 128).T.astype(np.float32)
        )

        r1a = np.zeros((2, NPT), np.float64)
        r1a[0] = 1.0 / disl
        r1a[1, :cnt] = kappa[n0:n1]
        m["r1a"] = np.ascontiguousarray(r1a.astype(BF16NP))

        # gather list, send-buffer slot order: dest o block, slot (t, p) ->
        # flat i = o*pair_cap + t*128 + p.
        gflat = np.zeros(cfg.sendcap, np.int16)
        for o in range(C):
            gflat[o * cfg.pair_cap : (o + 1) * cfg.pair_cap] = send_srcm[o][c].reshape(-1)
        m["gidx"] = _wrap_idx(gflat)

        # scatter list, wave-major: wave wv slots ordered (o, tloc, p).
        sl = []
        for wv, (a, b) in enumerate(tr):
            blk = recv_dst[c][:, a:b, :]  # [C, b-a, 128]
            sl.append(blk.reshape(-1))
        m["sidx"] = _wrap_idx(np.concatenate(sl))

        ph = np.zeros((NPT, 8), np.float64)
        lb = batch[n0:n1] - c * gpc
        ph[np.arange(cnt), lb] = 1.0
        m["poolh"] = np.ascontiguousarray(
            ph[: cfg.np_cap].reshape(NT, 128, 8).transpose(1, 0, 2).reshape(128, NT * 8).astype(BF16NP)
        )
        in_maps.append(m)

    return in_maps


def build(cfg: Cfg):
    import concourse.mybir as mybir
    import concourse.tile as tile
    from concourse import bacc
    from concourse.masks import make_identity

    dt = mybir.dt
    FP, BF, I16 = dt.float32, dt.bfloat16, dt.int16
    AF = mybir.ActivationFunctionType
    NPT, NT = cfg.npt, cfg.np_cap // 128
    C, TQ, NW = cfg.c, cfg.tq, cfg.nw
    SENDCAP = cfg.sendcap
    NIW = SENDCAP // 16
    widths = [DIMS[l][0] for l in range(5)]
    tr = cfg.wave_tr
    # split chunks' t-slots into half A = [0, TQA), half B = [TQA, TQ)
    TQA = TQ // 2
    TQB = TQ - TQA

    nc = bacc.Bacc("TRN2", target_bir_lowering=False, debug=False, num_devices=C)

    xt = nc.dram_tensor("xt", [NPT, 128], BF, kind="ExternalInput")
    gidx = nc.dram_tensor("gidx", [128, NIW], I16, kind="ExternalInput")
    sidx = nc.dram_tensor("sidx", [128, NIW], I16, kind="ExternalInput")
    disr = nc.dram_tensor("disr", [128, NT], FP, kind="ExternalInput")
    r1a = nc.dram_tensor("r1a", [2, NPT], BF, kind="ExternalInput")
    poolh = nc.dram_tensor("poolh", [128, NT * 8], BF, kind="ExternalInput")
    wd, r1bd = {}, {}
    for l in range(1, 6):
        din, dout = DIMS[l - 1]
        wd[l] = nc.dram_tensor(f"w{l}", [din, dout], BF, kind="ExternalInput")
        r1bd[l] = nc.dram_tensor(f"r1b{l}", [2, dout], BF, kind="ExternalInput")
    fw1 = nc.dram_tensor("fw1", [512, 512], FP, kind="ExternalInput")
    fb1 = nc.dram_tensor("fb1", [1, 512], FP, kind="ExternalInput")
    fw2 = nc.dram_tensor("fw2", [512, 1], FP, kind="ExternalInput")
    fb2 = nc.dram_tensor("fb2", [1, 1], FP, kind="ExternalInput")
    out_d = nc.dram_tensor("out", [8, 1], FP, kind="ExternalOutput")

    tabs = [nc.dram_tensor(f"t{l}", [NPT, widths[l - 1]], BF) for l in range(1, 6)]
    h5_d = nc.dram_tensor("h5", [NPT, 512], BF) if cfg.write_tables else None
    # a2a halves: flat layout = 8 chunks x [128, TQh, w] partition-major blocks
    a_inA = [nc.dram_tensor(f"a2ainA{l}", [C, 128, TQA, widths[l - 1]], BF) for l in range(1, 6)]
    a_inB = [nc.dram_tensor(f"a2ainB{l}", [C, 128, TQB, widths[l - 1]], BF) for l in range(1, 6)]
    a_outA = [nc.dram_tensor(f"a2aoutA{l}", [C, 128, TQA, widths[l - 1]], BF) for l in range(1, 6)]
    a_outB = [nc.dram_tensor(f"a2aoutB{l}", [C, 128, TQB, widths[l - 1]], BF) for l in range(1, 6)]

    with tile.TileContext(nc) as tc:
        with (
            tc.tile_pool(name="const", bufs=1) as const,
            tc.tile_pool(name="gat", bufs=2) as gpool,
            tc.tile_pool(name="sca", bufs=2) as spool,
            tc.tile_pool(name="zt", bufs=1) as zpool,
            tc.tile_pool(name="epi", bufs=3) as epool,
            tc.tile_pool(name="oute", bufs=3) as opool,
            tc.tile_pool(name="ps", bufs=4, space="PSUM") as ppool,
            tc.tile_pool(name="ps2", bufs=1, space="PSUM") as ppool2,
        ):
            gidx_sb = const.tile([128, NIW], I16)
            sidx_sb = const.tile([128, NIW], I16)
            disr_sb = const.tile([128, NT], FP)
            r1a_sb = const.tile([2, NPT], BF)
            poolh_sb = const.tile([128, NT, 8], BF)
            nc.sync.dma_start(gidx_sb[:], gidx[:, :])
            nc.sync.dma_start(sidx_sb[:], sidx[:, :])
            nc.sync.dma_start(disr_sb[:], disr[:, :])
            nc.sync.dma_start(r1a_sb[:], r1a[:, :])
            nc.sync.dma_start(poolh_sb[:], poolh[:, :].rearrange("p (t e) -> p t e", e=8))
            w_sb, r1b_sb = {}, {}
            for l in range(1, 6):
                din, dout = DIMS[l - 1]
                w_sb[l] = const.tile([128, din // 128, dout], BF, tag=f"w{l}", name=f"wsb{l}")
                nc.sync.dma_start(w_sb[l][:], wd[l][:, :].rearrange("(k p) n -> p k n", p=128))
                r1b_sb[l] = const.tile([2, dout], BF, tag=f"r1b{l}", name=f"r1bsb{l}")
                nc.sync.dma_start(r1b_sb[l][:], r1bd[l][:, :])
            fw1_sb = const.tile([128, 4, 512], FP)
            nc.sync.dma_start(fw1_sb[:], fw1[:, :].rearrange("(k p) n -> p k n", p=128))
            fw2_sb = const.tile([128, 4, 1], FP)
            nc.sync.dma_start(fw2_sb[:], fw2[:, :].rearrange("(k p) n -> p k n", p=128))
            fb1_sb = const.tile([1, 512], FP)
            nc.sync.dma_start(fb1_sb[:], fb1[:, :])
            fb2_sb = const.tile([1, 1], FP)
            nc.sync.dma_start(fb2_sb[:], fb2[:, :])
            ident_sb = const.tile([128, 128], FP)
            make_identity(nc, ident_sb[:])
            ones8_sb = const.tile([1, 8], FP)
            nc.vector.memset(ones8_sb[:], 1.0)

            nc.sync.dma_start(tabs[0][:, :], xt[:, :])
            zero_sb = const.tile([128, 512], BF)
            nc.vector.memset(zero_sb[:], 0.0)
            for li in range(1, 5):
                nc.sync.dma_start(tabs[li][cfg.np_cap : NPT, :], zero_sb[:, : widths[li]])

            pooled_ps = None
            for li in range(5):
                l = li + 1
                T = tabs[li]
                w = widths[li]
                din, dout = DIMS[li]
                K = din // 128

                # ---- Phase A: per-chunk gathers -> 2 AllToAlls -> waves ----
                # gather idx flat order: o*pair_cap + t*128 + p; half A of
                # chunk o = slots [o*pc, o*pc + TQA*128)
                def emit_gather(o, half):
                    t0, t1 = (0, TQA) if half == 0 else (TQA, TQ)
                    n = (t1 - t0) * 128
                    base = o * cfg.pair_cap + t0 * 128
                    g_t = gpool.tile([128, t1 - t0, w], BF, tag="gt", name="g_t")
                    nc.gpsimd.dma_gather(
                        out_ap=g_t[:],
                        in_ap=T[:, :],
                        idxs_ap=gidx_sb[:, base // 16 : (base + n) // 16],
                        num_idxs=n,
                        num_idxs_reg=n,
                        elem_size=w,
                        single_packet=False,
                    )
                    dest = a_inA[li] if half == 0 else a_inB[li]
                    nc.sync.dma_start(dest[o, :, :, :], g_t[:])

                for o in range(C):
                    emit_gather(o, 0)
                nc.gpsimd.collective_compute(
                    "AllToAll",
                    mybir.AluOpType.bypass,
                    replica_groups=[list(range(C))],
                    ins=[a_inA[li].ap().opt()],
                    outs=[a_outA[li].ap().opt()],
                )
                for o in range(C):
                    emit_gather(o, 1)
                nc.gpsimd.collective_compute(
                    "AllToAll",
                    mybir.AluOpType.bypass,
                    replica_groups=[list(range(C))],
                    ins=[a_inB[li].ap().opt()],
                    outs=[a_outB[li].ap().opt()],
                )

                # scatter waves: t-range (a, b) across all chunks
                spos = 0
                for wv, (a, b) in enumerate(tr):
                    n = (b - a) * 128 * C
                    s_t = spool.tile([128, C, b - a, w], BF, tag="st", name="s_t")
                    if b <= TQA:
                        srcv = a_outA[li][:, :, a:b, :]
                    elif a >= TQA:
                        srcv = a_outB[li][:, :, a - TQA : b - TQA, :]
                    else:
                        raise AssertionError("wave straddles halves")
                    # srcv [C, 128, ts, w] -> tile [128, C, ts, w]; per-chunk
                    # 3-level DMAs (a single 4-level AP hung on HW)
                    for o in range(C):
                        nc.sync.dma_start(s_t[:, o, :, :], srcv[o, :, :, :])
                    nc.gpsimd.dma_scatter_add(
                        out_ap=T[:, :],
                        in_ap=s_t[:].rearrange("p o t e -> p (o t) e"),
                        idxs_ap=sidx_sb[:, spos // 16 : (spos + n) // 16],
                        num_idxs=n,
                        num_idxs_reg=n,
                        elem_size=w,
                        single_packet=False,
                    )
                    spos += n

                # ---- Phase B: z^T load (halves), matmul + rank-1, epilogue ----
                if l == 5:
                    pooled_ps = ppool2.tile([128, 512], mybir.dt.float32, space="PSUM")
                NTH = NT // 2
                zT_halves = []
                for h in range(2):
                    r0, r1 = h * NTH * 128, (NTH * 128 if h == 0 else cfg.np_cap)
                    nth = (r1 - r0) // 128
                    zT = zpool.tile([128, K, nth * 128], BF, tag="zt", name="zT")
                    for kc in range(K):
                        nc.sync.dma_start_transpose(
                            zT[:, kc, :], T[r0:r1, kc * 128 : (kc + 1) * 128]
                        )
                    zT_halves.append((zT, r0 // 128, nth))
                for rt in range(NT):
                    zT, rt_base, _nth = zT_halves[0 if rt < NTH else 1]
                    rloc = rt - rt_base
                    ps = ppool.tile([128, dout], mybir.dt.float32, space="PSUM", tag="ps")
                    for kc in range(K):
                        nc.tensor.matmul(
                            ps[:],
                            lhsT=zT[:, kc, rloc * 128 : (rloc + 1) * 128],
                            rhs=w_sb[l][:, kc, :],
                            start=(kc == 0),
                            stop=False,
                        )
                    nc.tensor.matmul(
                        ps[:],
                        lhsT=r1a_sb[:, rt * 128 : (rt + 1) * 128],
                        rhs=r1b_sb[l][:],
                        start=False,
                        stop=True,
                    )
                    if l < 5:
                        e_t = epool.tile([128, dout], FP, tag="et")
                        nc.scalar.activation(
                            e_t[:], ps[:], AF.Relu, scale=disr_sb[:, rt : rt + 1]
                        )
                        o_t = opool.tile([128, dout], BF, tag="ot")
                        nc.vector.tensor_scalar_mul(o_t[:], e_t[:], disr_sb[:, rt : rt + 1])
                        nc.sync.dma_start(tabs[li + 1][rt * 128 : (rt + 1) * 128, :], o_t[:])
                    else:
                        o_t = opool.tile([128, dout], BF, tag="ot")
                        nc.scalar.activation(
                            o_t[:], ps[:], AF.Copy, scale=disr_sb[:, rt : rt + 1]
                        )
                        if cfg.write_tables:
                            nc.sync.dma_start(h5_d[rt * 128 : (rt + 1) * 128, :], o_t[:])
                        nc.tensor.matmul(
                            pooled_ps[:8, :],
                            lhsT=poolh_sb[:, rt, :],
                            rhs=o_t[:],
                            start=(rt == 0),
                            stop=(rt == NT - 1),
                        )

            # ---- head: relu -> fc1 -> relu -> fc2 ----
            pool_r = epool.tile([128, 512], FP, tag="pr")
            nc.scalar.activation(pool_r[:8, :], pooled_ps[:8, :], AF.Relu)
            pT = epool.tile([128, 4, 8], FP, tag="pT")
            for fc in range(4):
                tp = ppool.tile([128, 512], mybir.dt.float32, space="PSUM", tag="ps")
                nc.tensor.transpose(tp[:, :8], pool_r[:8, fc * 128 : (fc + 1) * 128], ident_sb[:8, :8])
                nc.vector.tensor_copy(pT[:, fc, :], tp[:, :8])
            hid_ps = ppool.tile([128, 512], mybir.dt.float32, space="PSUM", tag="ps")
            for fc in range(4):
                nc.tensor.matmul(
                    hid_ps[:8, :], lhsT=pT[:, fc, :], rhs=fw1_sb[:, fc, :],
                    start=(fc == 0), stop=False,
                )
            nc.tensor.matmul(hid_ps[:8, :], lhsT=ones8_sb[:1, :], rhs=fb1_sb[:1, :], start=False, stop=True)
            hid_r = epool.tile([128, 512], FP, tag="pr")
            nc.scalar.activation(hid_r[:8, :], hid_ps[:8, :], AF.Relu)
            hT = epool.tile([128, 4, 8], FP, tag="pT")
            for fc in range(4):
                tp = ppool.tile([128, 512], mybir.dt.float32, space="PSUM", tag="ps")
                nc.tensor.transpose(tp[:, :8], hid_r[:8, fc * 128 : (fc + 1) * 128], ident_sb[:8, :8])
                nc.vector.tensor_copy(hT[:, fc, :], tp[:, :8])
            out_ps = ppool.tile([128, 512], mybir.dt.float32, space="PSUM", tag="ps")
            for fc in range(4):
                nc.tensor.matmul(
                    out_ps[:8, :1], lhsT=hT[:, fc, :], rhs=fw2_sb[:, fc, :],
                    start=(fc == 0), stop=False,
                )
            nc.tensor.matmul(out_ps[:8, :1], lhsT=ones8_sb[:1, :], rhs=fb2_sb[:1, :], start=False, stop=True)
            out_sb = epool.tile([128, 1], FP, tag="osb")
            nc.vector.tensor_copy(out_sb[:8, :], out_ps[:8, :1])
            nc.sync.dma_start(out_d[:, :], out_sb[:8, :])

    nc.compile()
    return nc


_NC_CACHE = {}


def _get_nc(cfg: Cfg):
    if cfg not in _NC_CACHE:
        _NC_CACHE[cfg] = build(cfg)
    return _NC_CACHE[cfg]


def kernel(x, ei_intra, ei_inter, batch, params):
    cfg = FULL_CFG
    in_maps = preprocess(x, ei_intra, ei_inter, batch, params, cfg)
    nc = _get_nc(cfg)
    from concourse.bass_utils import run_bass_kernel_spmd

    res = run_bass_kernel_spmd(nc, in_maps, core_ids=list(range(cfg.c)))
    outs = [np.asarray(res.results[c]["out"], np.float32).reshape(8) for c in range(cfg.c)]
    return np.concatenate(outs)


# revision 18
# speedup vs baseline: 1.5606x; 1.5606x over previous
"""Trainium2 Bass kernel for nn_Atom3D (5-layer GCN + global pool + MLP head).

Distribution: graph-aligned node sharding across 8 NeuronCores (8 graphs/core).
Per layer: edge messages move via dma_gather (send-side row gather, per dest
chunk), two pipelined AllToAll collectives (half A / half B), and
dma_scatter_add (CCE accumulate) onto the local node table; the dense matmul +
folded-BN epilogue runs locally.

Layout: a2a buffers are partition-major blocks [128, TQ, w] per (dest) chunk so
bounce DMAs are big contiguous transfers.  The HW CCE scatter-add loses
concurrent adds to the same row, so scatter waves are t-slot ranges across all
8 chunks and the host assigns same-dst edges to distinct waves.

Math factoring (validated vs reference to ~4e-6):
  - GCN symmetric norm is separable: ne = dis[src]*dis[dst].  Tables store
    h~ = dis * h, messages are raw rows, and the dst-side dis multiplies the
    matmul *output* rows (diag row-scale commutes through z @ W).
  - BN (eval mode) scale s is folded into W columns (s > 0); bias b and the
    BN shift t enter PSUM as rank-1 matmuls (1/dis and kappa rows); the t of
    layers 1-3 is pushed through the linear aggregation into layer l+1 using
    kappa = A' @ dis (host-precomputed).
"""

from dataclasses import dataclass

import numpy as np
import ml_dtypes

BF16NP = ml_dtypes.bfloat16
BN_EPS = 1e-5
DIMS = [(128, 128), (128, 256), (256, 512), (512, 512), (512, 512)]


@dataclass(frozen=True)
class Cfg:
    n: int = 80000
    g: int = 64
    c: int = 8
    np_cap: int = 10240      # per-core node rows (multiple of 128)
    pair_cap: int = 2688     # per (src-core, dst-core) edge slots (mult of 128)
    nw: int = 12             # scatter waves (>= max in-degree + slack)
    write_tables: bool = False

    @property
    def npt(self) -> int:
        return self.np_cap + 128  # + trash tile (dummy-edge scatter target)

    @property
    def sendcap(self) -> int:
        return self.c * self.pair_cap

    @property
    def tq(self) -> int:
        assert self.pair_cap % 128 == 0
        return self.pair_cap // 128  # t-slots per chunk

    @property
    def wave_tr(self) -> list[tuple[int, int]]:
        """Scatter-wave t-ranges [(t0, t1)) covering 0..TQ, nw of them."""
        tq, nw = self.tq, self.nw
        base, extra = divmod(tq, nw)
        out = []
        t = 0
        for i in range(nw):
            sz = base + (1 if i < extra else 0)
            out.append((t, t + sz))
            t += sz
        assert t == tq and all(b > a for a, b in out)
        return out


FULL_CFG = Cfg()


def _wrap_idx(ids: np.ndarray) -> np.ndarray:
    """int16 index array -> [128, len/16] wrapped layout (16-partition pattern
    replicated 8x for the Q7 cores): idx i lives at [i % 16 (+16k), i // 16]."""
    assert len(ids) % 16 == 0
    return np.ascontiguousarray(np.tile(ids.reshape(-1, 16).T, (8, 1)).astype(np.int16))


def preprocess(x, ei_intra, ei_inter, batch, params, cfg: Cfg):
    """All-numpy host prep. Returns in_maps (one dict per core)."""
    N, G, C = cfg.n, cfg.g, cfg.c
    gpc = G // C
    x = np.asarray(x, np.float64)
    batch = np.asarray(batch)
    src = np.concatenate([np.asarray(ei_intra)[0], np.asarray(ei_inter)[0]])
    dst = np.concatenate([np.asarray(ei_intra)[1], np.asarray(ei_inter)[1]])

    deg = np.bincount(dst, minlength=N).astype(np.float64) + 1.0
    dis = 1.0 / np.sqrt(deg)
    kappa = np.bincount(dst, weights=dis[src], minlength=N) + dis

    gcounts = np.bincount(batch, minlength=G)
    gstart = np.concatenate([[0], np.cumsum(gcounts)]).astype(np.int64)
    core_start = gstart[0 : G : gpc]
    core_end = gstart[gpc :: gpc]
    ncnt = core_end - core_start
    assert ncnt.max() <= cfg.np_cap, (ncnt.max(), cfg.np_cap)
    core_of = np.zeros(N, np.int64)
    for c in range(C):
        core_of[core_start[c] : core_end[c]] = c

    p = {k: np.asarray(v, np.float64) for k, v in params.items()}
    s, t, Wf, bt, tau = {}, {}, {}, {}, {}
    for l in range(1, 6):
        s[l] = p[f"g{l}"] / np.sqrt(p[f"v{l}"] + BN_EPS)
        assert (s[l] > 0).all(), "BN scale sign flip not supported"
        t[l] = p[f"beta{l}"] - p[f"m{l}"] * s[l]
        Wf[l] = p[f"W{l}"] * s[l][None, :]
    for l in range(1, 6):
        bt[l] = s[l] * p[f"b{l}"]
        if l in (4, 5):
            bt[l] = bt[l] + t[l]
        tau[l] = t[l - 1] @ Wf[l] if l in (2, 3, 4) else np.zeros(DIMS[l - 1][1])

    score = core_of[src]
    dcore = core_of[dst]
    pair_g = [[None] * C for _ in range(C)]  # [o][c] sender-local src rows
    pair_s = [[None] * C for _ in range(C)]  # [o][c] receiver-local dst rows
    for o in range(C):
        mo = score == o
        for c in range(C):
            e = np.nonzero(mo & (dcore == c))[0]
            assert len(e) <= cfg.pair_cap, (o, c, len(e), cfg.pair_cap)
            pair_g[o][c] = (src[e] - core_start[o]).astype(np.int16)
            pair_s[o][c] = (dst[e] - core_start[c]).astype(np.int16)

    NPT, NT = cfg.npt, cfg.np_cap // 128
    TRASH = cfg.np_cap
    NW, TQ = cfg.nw, cfg.tq
    tr = cfg.wave_tr
    wave_of_t = np.zeros(TQ, np.int64)
    for wv, (a, b) in enumerate(tr):
        wave_of_t[a:b] = wv
    slots_per_wave = [(b - a) * 128 for a, b in tr]

    # Assign each receiver's edges to (chunk o, t-slot, p): same-dst edges in
    # distinct waves (t-ranges). Slot (o, t, p) <-> chunk-local pos t*128+p.
    recv_dst = []   # [c] -> [C, TQ, 128] receiver-local dst (TRASH = dummy)
    send_srcm = []  # [c] -> [C, TQ, 128] sender(o)-local src rows (0 = dummy)
    from collections import defaultdict

    for c in range(C):
        by_dst = defaultdict(list)
        for o in range(C):
            for srcl, dstl in zip(pair_g[o][c], pair_s[o][c]):
                by_dst[int(dstl)].append((o, int(srcl)))
        fill = np.zeros((C, NW), np.int64)  # slots used per (chunk, wave)
        sdst = np.full((C, TQ, 128), TRASH, np.int16)
        ssrc = np.zeros((C, TQ, 128), np.int16)
        for d, elist in sorted(by_dst.items(), key=lambda kv: -len(kv[1])):
            used = set()
            for o, srcl in elist:
                cands = [
                    w for w in range(NW)
                    if w not in used and fill[o][w] < slots_per_wave[w]
                ]
                assert cands, f"wave assignment infeasible (core {c}, dst {d})"
                w = min(cands, key=lambda ww: fill[o][ww] / slots_per_wave[ww])
                k = fill[o][w]
                fill[o][w] += 1
                t0 = tr[w][0] + k // 128
                sdst[o, t0, k % 128] = d
                ssrc[o, t0, k % 128] = srcl
                used.add(w)
        recv_dst.append(sdst)
        send_srcm.append(ssrc)

    def shared_tensors():
        m = {}
        for l in range(1, 6):
            m[f"w{l}"] = np.ascontiguousarray(Wf[l].astype(BF16NP))
            m[f"r1b{l}"] = np.ascontiguousarray(np.stack([bt[l], tau[l]]).astype(BF16NP))
        m["fw1"] = np.ascontiguousarray(p["fcW1"].astype(np.float32))
        m["fb1"] = np.ascontiguousarray(p["fcb1"].astype(np.float32)[None, :])
        m["fw2"] = np.ascontiguousarray(p["fcW2"].astype(np.float32))
        m["fb2"] = np.ascontiguousarray(p["fcb2"].astype(np.float32)[None, :])
        return m

    shared = shared_tensors()
    in_maps = []
    for c in range(C):
        n0, n1 = core_start[c], core_end[c]
        cnt = n1 - n0
        m = dict(shared)

        xt = np.zeros((NPT, 128), np.float64)
        xt[:cnt] = dis[n0:n1, None] * x[n0:n1]
        m["xt"] = np.ascontiguousarray(xt.astype(BF16NP))

        disl = np.ones(NPT, np.float64)
        disl[:cnt] = dis[n0:n1]
        m["disr"] = np.ascontiguousarray(
            disl[: cfg.np_cap].reshape(NT, 128).T.astype(np.float32)
        )

        r1a = np.zeros((2, NPT), np.float64)
        r1a[0] = 1.0 / disl
        r1a[1, :cnt] = kappa[n0:n1]
        m["r1a"] = np.ascontiguousarray(r1a.astype(BF16NP))

        # gather list, send-buffer slot order: dest o block, slot (t, p) ->
        # flat i = o*pair_cap + t*128 + p.
        gflat = np.zeros(cfg.sendcap, np.int16)
        for o in range(C):
            gflat[o * cfg.pair_cap : (o + 1) * cfg.pair_cap] = send_srcm[o][c].reshape(-1)
        m["gidx"] = _wrap_idx(gflat)

        # scatter list, wave-major: wave wv slots ordered (o, tloc, p).
        sl = []
        for wv, (a, b) in enumerate(tr):
            blk = recv_dst[c][:, a:b, :]  # [C, b-a, 128]
            sl.append(blk.reshape(-1))
        m["sidx"] = _wrap_idx(np.concatenate(sl))

        ph = np.zeros((NPT, 8), np.float64)
        lb = batch[n0:n1] - c * gpc
        ph[np.arange(cnt), lb] = 1.0
        m["poolh"] = np.ascontiguousarray(
            ph[: cfg.np_cap].reshape(NT, 128, 8).transpose(1, 0, 2).reshape(128, NT * 8).astype(BF16NP)
        )
        in_maps.append(m)

    return in_maps


def build(cfg: Cfg):
    import concourse.mybir as mybir
    import concourse.tile as tile
    from concourse import bacc
    from concourse.masks import make_identity

    dt = mybir.dt
    FP, BF, I16 = dt.float32, dt.bfloat16, dt.int16
    AF = mybir.ActivationFunctionType
    NPT, NT = cfg.npt, cfg.np_cap // 128
    C, TQ, NW = cfg.c, cfg.tq, cfg.nw
    SENDCAP = cfg.sendcap
    NIW = SENDCAP // 16
    widths = [DIMS[l][0] for l in range(5)]
    tr = cfg.wave_tr
    # split chunks' t-slots at a wave boundary: half A = [0, TQA), B = rest
    TQA = tr[len(tr) // 2][0]
    TQB = TQ - TQA

    nc = bacc.Bacc("TRN2", target_bir_lowering=False, debug=False, num_devices=C)

    xt = nc.dram_tensor("xt", [NPT, 128], BF, kind="ExternalInput")
    gidx = nc.dram_tensor("gidx", [128, NIW], I16, kind="ExternalInput")
    sidx = nc.dram_tensor("sidx", [128, NIW], I16, kind="ExternalInput")
    disr = nc.dram_tensor("disr", [128, NT], FP, kind="ExternalInput")
    r1a = nc.dram_tensor("r1a", [2, NPT], BF, kind="ExternalInput")
    poolh = nc.dram_tensor("poolh", [128, NT * 8], BF, kind="ExternalInput")
    wd, r1bd = {}, {}
    for l in range(1, 6):
        din, dout = DIMS[l - 1]
        wd[l] = nc.dram_tensor(f"w{l}", [din, dout], BF, kind="ExternalInput")
        r1bd[l] = nc.dram_tensor(f"r1b{l}", [2, dout], BF, kind="ExternalInput")
    fw1 = nc.dram_tensor("fw1", [512, 512], FP, kind="ExternalInput")
    fb1 = nc.dram_tensor("fb1", [1, 512], FP, kind="ExternalInput")
    fw2 = nc.dram_tensor("fw2", [512, 1], FP, kind="ExternalInput")
    fb2 = nc.dram_tensor("fb2", [1, 1], FP, kind="ExternalInput")
    out_d = nc.dram_tensor("out", [8, 1], FP, kind="ExternalOutput")

    tabs = [nc.dram_tensor(f"t{l}", [NPT, widths[l - 1]], BF) for l in range(1, 6)]
    h5_d = nc.dram_tensor("h5", [NPT, 512], BF) if cfg.write_tables else None
    # a2a halves: flat layout = 8 chunks x [128, TQh, w] partition-major blocks
    a_inA = [nc.dram_tensor(f"a2ainA{l}", [C, 128, TQA, widths[l - 1]], BF) for l in range(1, 6)]
    a_inB = [nc.dram_tensor(f"a2ainB{l}", [C, 128, TQB, widths[l - 1]], BF) for l in range(1, 6)]
    a_outA = [nc.dram_tensor(f"a2aoutA{l}", [C, 128, TQA, widths[l - 1]], BF) for l in range(1, 6)]
    a_outB = [nc.dram_tensor(f"a2aoutB{l}", [C, 128, TQB, widths[l - 1]], BF) for l in range(1, 6)]

    with tile.TileContext(nc) as tc:
        with (
            tc.tile_pool(name="const", bufs=1) as const,
            tc.tile_pool(name="gat", bufs=2) as gpool,
            tc.tile_pool(name="sca", bufs=2) as spool,
            tc.tile_pool(name="zt", bufs=2) as zpool,
            tc.tile_pool(name="epi", bufs=4) as epool,
            tc.tile_pool(name="oute", bufs=4) as opool,
            tc.tile_pool(name="ps", bufs=6, space="PSUM") as ppool,
            tc.tile_pool(name="ps2", bufs=1, space="PSUM") as ppool2,
        ):
            gidx_sb = const.tile([128, NIW], I16)
            sidx_sb = const.tile([128, NIW], I16)
            disr_sb = const.tile([128, NT], FP)
            r1a_sb = const.tile([2, NPT], BF)
            poolh_sb = const.tile([128, NT, 8], BF)
            nc.sync.dma_start(gidx_sb[:], gidx[:, :])
            nc.sync.dma_start(sidx_sb[:], sidx[:, :])
            nc.sync.dma_start(disr_sb[:], disr[:, :])
            nc.sync.dma_start(r1a_sb[:], r1a[:, :])
            nc.sync.dma_start(poolh_sb[:], poolh[:, :].rearrange("p (t e) -> p t e", e=8))
            w_sb, r1b_sb = {}, {}
            for l in range(1, 6):
                din, dout = DIMS[l - 1]
                w_sb[l] = const.tile([128, din // 128, dout], BF, tag=f"w{l}", name=f"wsb{l}")
                nc.sync.dma_start(w_sb[l][:], wd[l][:, :].rearrange("(k p) n -> p k n", p=128))
                r1b_sb[l] = const.tile([2, dout], BF, tag=f"r1b{l}", name=f"r1bsb{l}")
                nc.sync.dma_start(r1b_sb[l][:], r1bd[l][:, :])
            fw1_sb = const.tile([128, 4, 512], FP)
            nc.sync.dma_start(fw1_sb[:], fw1[:, :].rearrange("(k p) n -> p k n", p=128))
            fw2_sb = const.tile([128, 4, 1], FP)
            nc.sync.dma_start(fw2_sb[:], fw2[:, :].rearrange("(k p) n -> p k n", p=128))
            fb1_sb = const.tile([1, 512], FP)
            nc.sync.dma_start(fb1_sb[:], fb1[:, :])
            fb2_sb = const.tile([1, 1], FP)
            nc.sync.dma_start(fb2_sb[:], fb2[:, :])
            ident_sb = const.tile([128, 128], FP)
            make_identity(nc, ident_sb[:])
            ones8_sb = const.tile([1, 8], FP)
            nc.vector.memset(ones8_sb[:], 1.0)

            nc.sync.dma_start(tabs[0][:, :], xt[:, :])
            zero_sb = const.tile([128, 512], BF)
            nc.vector.memset(zero_sb[:], 0.0)
            for li in range(1, 5):
                nc.sync.dma_start(tabs[li][cfg.np_cap : NPT, :], zero_sb[:, : widths[li]])

            pooled_ps = None
            for li in range(5):
                l = li + 1
                T = tabs[li]
                w = widths[li]
                din, dout = DIMS[li]
                K = din // 128

                # ---- Phase A: per-chunk gathers -> 2 AllToAlls -> waves ----
                # gather idx flat order: o*pair_cap + t*128 + p; half A of
                # chunk o = slots [o*pc, o*pc + TQA*128)
                def emit_gather(o, half):
                    t0, t1 = (0, TQA) if half == 0 else (TQA, TQ)
                    n = (t1 - t0) * 128
                    base = o * cfg.pair_cap + t0 * 128
                    g_t = gpool.tile([128, t1 - t0, w], BF, tag="gt", name="g_t")
                    nc.gpsimd.dma_gather(
                        out_ap=g_t[:],
                        in_ap=T[:, :],
                        idxs_ap=gidx_sb[:, base // 16 : (base + n) // 16],
                        num_idxs=n,
                        num_idxs_reg=n,
                        elem_size=w,
                        single_packet=False,
                    )
                    dest = a_inA[li] if half == 0 else a_inB[li]
                    nc.sync.dma_start(dest[o, :, :, :], g_t[:])

                for o in range(C):
                    emit_gather(o, 0)
                nc.gpsimd.collective_compute(
                    "AllToAll",
                    mybir.AluOpType.bypass,
                    replica_groups=[list(range(C))],
                    ins=[a_inA[li].ap().opt()],
                    outs=[a_outA[li].ap().opt()],
                )
                for o in range(C):
                    emit_gather(o, 1)
                nc.gpsimd.collective_compute(
                    "AllToAll",
                    mybir.AluOpType.bypass,
                    replica_groups=[list(range(C))],
                    ins=[a_inB[li].ap().opt()],
                    outs=[a_outB[li].ap().opt()],
                )

                # scatter waves: t-range (a, b) across all chunks
                spos = 0
                for wv, (a, b) in enumerate(tr):
                    n = (b - a) * 128 * C
                    s_t = spool.tile([128, C, b - a, w], BF, tag="st", name="s_t")
                    if b <= TQA:
                        srcv = a_outA[li][:, :, a:b, :]
                    elif a >= TQA:
                        srcv = a_outB[li][:, :, a - TQA : b - TQA, :]
                    else:
                        raise AssertionError("wave straddles halves")
                    # srcv [C, 128, ts, w] -> tile [128, C, ts, w]; per-chunk
                    # 3-level DMAs (a single 4-level AP hung on HW)
                    for o in range(C):
                        nc.sync.dma_start(s_t[:, o, :, :], srcv[o, :, :, :])
                    nc.gpsimd.dma_scatter_add(
                        out_ap=T[:, :],
                        in_ap=s_t[:].rearrange("p o t e -> p (o t) e"),
                        idxs_ap=sidx_sb[:, spos // 16 : (spos + n) // 16],
                        num_idxs=n,
                        num_idxs_reg=n,
                        elem_size=w,
                        single_packet=False,
                    )
                    spos += n

                # ---- Phase B: z^T load (quarters, pipelined), matmul ----
                if l == 5:
                    pooled_ps = ppool2.tile([128, 512], mybir.dt.float32, space="PSUM")
                NQ = min(4, NT)
                NTH = NT // NQ
                zT_parts = []
                for h in range(NQ):
                    r0 = h * NTH * 128
                    r1 = (h + 1) * NTH * 128 if h < NQ - 1 else cfg.np_cap
                    nth = (r1 - r0) // 128
                    zT = zpool.tile([128, K, nth * 128], BF, tag="zt", name="zT")
                    for kc in range(K):
                        nc.sync.dma_start_transpose(
                            zT[:, kc, :], T[r0:r1, kc * 128 : (kc + 1) * 128]
                        )
                    zT_parts.append((zT, r0 // 128, nth))
                for rt in range(NT):
                    zT, rt_base, _nth = zT_parts[min(rt // NTH, NQ - 1)]
                    rloc = rt - rt_base
                    ps = ppool.tile([128, dout], mybir.dt.float32, space="PSUM", tag="ps")
                    for kc in range(K):
                        nc.tensor.matmul(
                            ps[:],
                            lhsT=zT[:, kc, rloc * 128 : (rloc + 1) * 128],
                            rhs=w_sb[l][:, kc, :],
                            start=(kc == 0),
                            stop=False,
                        )
                    nc.tensor.matmul(
                        ps[:],
                        lhsT=r1a_sb[:, rt * 128 : (rt + 1) * 128],
                        rhs=r1b_sb[l][:],
                        start=False,
                        stop=True,
                    )
                    if l < 5:
                        e_t = epool.tile([128, dout], FP, tag="et")
                        nc.scalar.activation(
                            e_t[:], ps[:], AF.Relu, scale=disr_sb[:, rt : rt + 1]
                        )
                        o_t = opool.tile([128, dout], BF, tag="ot")
                        nc.vector.tensor_scalar_mul(o_t[:], e_t[:], disr_sb[:, rt : rt + 1])
                        nc.sync.dma_start(tabs[li + 1][rt * 128 : (rt + 1) * 128, :], o_t[:])
                    else:
                        o_t = opool.tile([128, dout], BF, tag="ot")
                        nc.scalar.activation(
                            o_t[:], ps[:], AF.Copy, scale=disr_sb[:, rt : rt + 1]
                        )
                        if cfg.write_tables:
                            nc.sync.dma_start(h5_d[rt * 128 : (rt + 1) * 128, :], o_t[:])
                        nc.tensor.matmul(
                            pooled_ps[:8, :],
                            lhsT=poolh_sb[:, rt, :],
                            rhs=o_t[:],
                            start=(rt == 0),
                            stop=(rt == NT - 1),
                        )

            # ---- head: relu -> fc1 -> relu -> fc2 ----
            pool_r = epool.tile([128, 512], FP, tag="pr")
            nc.scalar.activation(pool_r[:8, :], pooled_ps[:8, :], AF.Relu)
            pT = epool.tile([128, 4, 8], FP, tag="pT")
            for fc in range(4):
                tp = ppool.tile([128, 512], mybir.dt.float32, space="PSUM", tag="ps")
                nc.tensor.transpose(tp[:, :8], pool_r[:8, fc * 128 : (fc + 1) * 128], ident_sb[:8, :8])
                nc.vector.tensor_copy(pT[:, fc, :], tp[:, :8])
            hid_ps = ppool.tile([128, 512], mybir.dt.float32, space="PSUM", tag="ps")
            for fc in range(4):
                nc.tensor.matmul(
                    hid_ps[:8, :], lhsT=pT[:, fc, :], rhs=fw1_sb[:, fc, :],
                    start=(fc == 0), stop=False,
                )
            nc.tensor.matmul(hid_ps[:8, :], lhsT=ones8_sb[:1, :], rhs=fb1_sb[:1, :], start=False, stop=True)
            hid_r = epool.tile([128, 512], FP, tag="pr")
            nc.scalar.activation(hid_r[:8, :], hid_ps[:8, :], AF.Relu)
            hT = epool.tile([128, 4, 8], FP, tag="pT")
            for fc in range(4):
                tp = ppool.tile([128, 512], mybir.dt.float32, space="PSUM", tag="ps")
                nc.tensor.transpose(tp[:, :8], hid_r[:8, fc * 128 : (fc + 1) * 128], ident_sb[:8, :8])
                nc.vector.tensor_copy(hT[:, fc, :], tp[:, :8])
            out_ps = ppool.tile([128, 512], mybir.dt.float32, space="PSUM", tag="ps")
            for fc in range(4):
                nc.tensor.matmul(
                    out_ps[:8, :1], lhsT=hT[:, fc, :], rhs=fw2_sb[:, fc, :],
                    start=(fc == 0), stop=False,
                )
            nc.tensor.matmul(out_ps[:8, :1], lhsT=ones8_sb[:1, :], rhs=fb2_sb[:1, :], start=False, stop=True)
            out_sb = epool.tile([128, 1], FP, tag="osb")
            nc.vector.tensor_copy(out_sb[:8, :], out_ps[:8, :1])
            nc.sync.dma_start(out_d[:, :], out_sb[:8, :])

    nc.compile()
    return nc


_NC_CACHE = {}


def _get_nc(cfg: Cfg):
    if cfg not in _NC_CACHE:
        _NC_CACHE[cfg] = build(cfg)
    return _NC_CACHE[cfg]


def kernel(x, ei_intra, ei_inter, batch, params):
    cfg = FULL_CFG
    in_maps = preprocess(x, ei_intra, ei_inter, batch, params, cfg)
    nc = _get_nc(cfg)
    from concourse.bass_utils import run_bass_kernel_spmd

    res = run_bass_kernel_spmd(nc, in_maps, core_ids=list(range(cfg.c)))
    outs = [np.asarray(res.results[c]["out"], np.float32).reshape(8) for c in range(cfg.c)]
    return np.concatenate(outs)


# revision 19
# speedup vs baseline: 1.6644x; 1.0665x over previous
"""Trainium2 Bass kernel for nn_Atom3D (5-layer GCN + global pool + MLP head).

Distribution: graph-aligned node sharding across 8 NeuronCores (8 graphs/core).
Per layer: edge messages move via dma_gather (send-side row gather, per dest
chunk), two pipelined AllToAll collectives (half A / half B), and
dma_scatter_add (CCE accumulate) onto the local node table; the dense matmul +
folded-BN epilogue runs locally.

Layout: a2a buffers are partition-major blocks [128, TQ, w] per (dest) chunk so
bounce DMAs are big contiguous transfers.  The HW CCE scatter-add loses
concurrent adds to the same row, so scatter waves are t-slot ranges across all
8 chunks and the host assigns same-dst edges to distinct waves.

Math factoring (validated vs reference to ~4e-6):
  - GCN symmetric norm is separable: ne = dis[src]*dis[dst].  Tables store
    h~ = dis * h, messages are raw rows, and the dst-side dis multiplies the
    matmul *output* rows (diag row-scale commutes through z @ W).
  - BN (eval mode) scale s is folded into W columns (s > 0); bias b and the
    BN shift t enter PSUM as rank-1 matmuls (1/dis and kappa rows); the t of
    layers 1-3 is pushed through the linear aggregation into layer l+1 using
    kappa = A' @ dis (host-precomputed).
"""

from dataclasses import dataclass

import numpy as np
import ml_dtypes

BF16NP = ml_dtypes.bfloat16
BN_EPS = 1e-5
DIMS = [(128, 128), (128, 256), (256, 512), (512, 512), (512, 512)]


@dataclass(frozen=True)
class Cfg:
    n: int = 80000
    g: int = 64
    c: int = 8
    np_cap: int = 10240      # per-core node rows (multiple of 128)
    pair_cap: int = 2688     # per (src-core, dst-core) edge slots (mult of 128)
    nw: int = 12             # scatter waves (>= max in-degree + slack)
    write_tables: bool = False

    @property
    def npt(self) -> int:
        return self.np_cap + 128  # + trash tile (dummy-edge scatter target)

    @property
    def sendcap(self) -> int:
        return self.c * self.pair_cap

    @property
    def tq(self) -> int:
        assert self.pair_cap % 128 == 0
        return self.pair_cap // 128  # t-slots per chunk

    @property
    def wave_tr(self) -> list[tuple[int, int]]:
        """Scatter-wave t-ranges [(t0, t1)) covering 0..TQ, nw of them."""
        tq, nw = self.tq, self.nw
        base, extra = divmod(tq, nw)
        out = []
        t = 0
        for i in range(nw):
            sz = base + (1 if i < extra else 0)
            out.append((t, t + sz))
            t += sz
        assert t == tq and all(b > a for a, b in out)
        return out


FULL_CFG = Cfg()


def _wrap_idx(ids: np.ndarray) -> np.ndarray:
    """int16 index array -> [128, len/16] wrapped layout (16-partition pattern
    replicated 8x for the Q7 cores): idx i lives at [i % 16 (+16k), i // 16]."""
    assert len(ids) % 16 == 0
    return np.ascontiguousarray(np.tile(ids.reshape(-1, 16).T, (8, 1)).astype(np.int16))


def preprocess(x, ei_intra, ei_inter, batch, params, cfg: Cfg):
    """All-numpy host prep. Returns in_maps (one dict per core)."""
    N, G, C = cfg.n, cfg.g, cfg.c
    gpc = G // C
    x = np.asarray(x, np.float64)
    batch = np.asarray(batch)
    src = np.concatenate([np.asarray(ei_intra)[0], np.asarray(ei_inter)[0]])
    dst = np.concatenate([np.asarray(ei_intra)[1], np.asarray(ei_inter)[1]])

    deg = np.bincount(dst, minlength=N).astype(np.float64) + 1.0
    dis = 1.0 / np.sqrt(deg)
    kappa = np.bincount(dst, weights=dis[src], minlength=N) + dis

    gcounts = np.bincount(batch, minlength=G)
    gstart = np.concatenate([[0], np.cumsum(gcounts)]).astype(np.int64)
    core_start = gstart[0 : G : gpc]
    core_end = gstart[gpc :: gpc]
    ncnt = core_end - core_start
    assert ncnt.max() <= cfg.np_cap, (ncnt.max(), cfg.np_cap)
    core_of = np.zeros(N, np.int64)
    for c in range(C):
        core_of[core_start[c] : core_end[c]] = c

    p = {k: np.asarray(v, np.float64) for k, v in params.items()}
    s, t, Wf, bt, tau = {}, {}, {}, {}, {}
    for l in range(1, 6):
        s[l] = p[f"g{l}"] / np.sqrt(p[f"v{l}"] + BN_EPS)
        assert (s[l] > 0).all(), "BN scale sign flip not supported"
        t[l] = p[f"beta{l}"] - p[f"m{l}"] * s[l]
        Wf[l] = p[f"W{l}"] * s[l][None, :]
    for l in range(1, 6):
        bt[l] = s[l] * p[f"b{l}"]
        if l in (4, 5):
            bt[l] = bt[l] + t[l]
        tau[l] = t[l - 1] @ Wf[l] if l in (2, 3, 4) else np.zeros(DIMS[l - 1][1])

    score = core_of[src]
    dcore = core_of[dst]
    pair_g = [[None] * C for _ in range(C)]  # [o][c] sender-local src rows
    pair_s = [[None] * C for _ in range(C)]  # [o][c] receiver-local dst rows
    for o in range(C):
        mo = score == o
        for c in range(C):
            e = np.nonzero(mo & (dcore == c))[0]
            assert len(e) <= cfg.pair_cap, (o, c, len(e), cfg.pair_cap)
            pair_g[o][c] = (src[e] - core_start[o]).astype(np.int16)
            pair_s[o][c] = (dst[e] - core_start[c]).astype(np.int16)

    NPT, NT = cfg.npt, cfg.np_cap // 128
    TRASH = cfg.np_cap
    NW, TQ = cfg.nw, cfg.tq
    tr = cfg.wave_tr
    wave_of_t = np.zeros(TQ, np.int64)
    for wv, (a, b) in enumerate(tr):
        wave_of_t[a:b] = wv
    slots_per_wave = [(b - a) * 128 for a, b in tr]

    # Assign each receiver's edges to (chunk o, t-slot, p): same-dst edges in
    # distinct waves (t-ranges). Slot (o, t, p) <-> chunk-local pos t*128+p.
    recv_dst = []   # [c] -> [C, TQ, 128] receiver-local dst (TRASH = dummy)
    send_srcm = []  # [c] -> [C, TQ, 128] sender(o)-local src rows (0 = dummy)
    from collections import defaultdict

    for c in range(C):
        by_dst = defaultdict(list)
        for o in range(C):
            for srcl, dstl in zip(pair_g[o][c], pair_s[o][c]):
                by_dst[int(dstl)].append((o, int(srcl)))
        fill = np.zeros((C, NW), np.int64)  # slots used per (chunk, wave)
        sdst = np.full((C, TQ, 128), TRASH, np.int16)
        ssrc = np.zeros((C, TQ, 128), np.int16)
        for d, elist in sorted(by_dst.items(), key=lambda kv: -len(kv[1])):
            used = set()
            for o, srcl in elist:
                cands = [
                    w for w in range(NW)
                    if w not in used and fill[o][w] < slots_per_wave[w]
                ]
                assert cands, f"wave assignment infeasible (core {c}, dst {d})"
                w = min(cands, key=lambda ww: fill[o][ww] / slots_per_wave[ww])
                k = fill[o][w]
                fill[o][w] += 1
                t0 = tr[w][0] + k // 128
                sdst[o, t0, k % 128] = d
                ssrc[o, t0, k % 128] = srcl
                used.add(w)
        recv_dst.append(sdst)
        send_srcm.append(ssrc)

    def shared_tensors():
        m = {}
        for l in range(1, 6):
            m[f"w{l}"] = np.ascontiguousarray(Wf[l].astype(BF16NP))
            m[f"r1b{l}"] = np.ascontiguousarray(np.stack([bt[l], tau[l]]).astype(BF16NP))
        m["fw1"] = np.ascontiguousarray(p["fcW1"].astype(np.float32))
        m["fb1"] = np.ascontiguousarray(p["fcb1"].astype(np.float32)[None, :])
        m["fw2"] = np.ascontiguousarray(p["fcW2"].astype(np.float32))
        m["fb2"] = np.ascontiguousarray(p["fcb2"].astype(np.float32)[None, :])
        return m

    shared = shared_tensors()
    in_maps = []
    for c in range(C):
        n0, n1 = core_start[c], core_end[c]
        cnt = n1 - n0
        m = dict(shared)

        xt = np.zeros((NPT, 128), np.float64)
        xt[:cnt] = dis[n0:n1, None] * x[n0:n1]
        m["xt"] = np.ascontiguousarray(xt.astype(BF16NP))

        disl = np.ones(NPT, np.float64)
        disl[:cnt] = dis[n0:n1]
        m["disr"] = np.ascontiguousarray(
            disl[: cfg.np_cap].reshape(NT, 128).T.astype(np.float32)
        )

        r1a = np.zeros((2, NPT), np.float64)
        r1a[0] = 1.0 / disl
        r1a[1, :cnt] = kappa[n0:n1]
        m["r1a"] = np.ascontiguousarray(r1a.astype(BF16NP))

        # gather list, send-buffer slot order: dest o block, slot (t, p) ->
        # flat i = o*pair_cap + t*128 + p.
        gflat = np.zeros(cfg.sendcap, np.int16)
        for o in range(C):
            gflat[o * cfg.pair_cap : (o + 1) * cfg.pair_cap] = send_srcm[o][c].reshape(-1)
        m["gidx"] = _wrap_idx(gflat)

        # scatter list, wave-major: wave wv slots ordered (o, tloc, p).
        sl = []
        for wv, (a, b) in enumerate(tr):
            blk = recv_dst[c][:, a:b, :]  # [C, b-a, 128]
            sl.append(blk.reshape(-1))
        m["sidx"] = _wrap_idx(np.concatenate(sl))

        ph = np.zeros((NPT, 8), np.float64)
        lb = batch[n0:n1] - c * gpc
        ph[np.arange(cnt), lb] = 1.0
        m["poolh"] = np.ascontiguousarray(
            ph[: cfg.np_cap].reshape(NT, 128, 8).transpose(1, 0, 2).reshape(128, NT * 8).astype(BF16NP)
        )
        in_maps.append(m)

    return in_maps


def build(cfg: Cfg):
    import concourse.mybir as mybir
    import concourse.tile as tile
    from concourse import bacc
    from concourse.masks import make_identity

    dt = mybir.dt
    FP, BF, I16 = dt.float32, dt.bfloat16, dt.int16
    AF = mybir.ActivationFunctionType
    NPT, NT = cfg.npt, cfg.np_cap // 128
    C, TQ, NW = cfg.c, cfg.tq, cfg.nw
    SENDCAP = cfg.sendcap
    NIW = SENDCAP // 16
    widths = [DIMS[l][0] for l in range(5)]
    tr = cfg.wave_tr
    # split chunks' t-slots at a wave boundary: half A = [0, TQA), B = rest
    TQA = tr[len(tr) // 2][0]
    TQB = TQ - TQA

    nc = bacc.Bacc("TRN2", target_bir_lowering=False, debug=False, num_devices=C)

    xt = nc.dram_tensor("xt", [NPT, 128], BF, kind="ExternalInput")
    gidx = nc.dram_tensor("gidx", [128, NIW], I16, kind="ExternalInput")
    sidx = nc.dram_tensor("sidx", [128, NIW], I16, kind="ExternalInput")
    disr = nc.dram_tensor("disr", [128, NT], FP, kind="ExternalInput")
    r1a = nc.dram_tensor("r1a", [2, NPT], BF, kind="ExternalInput")
    poolh = nc.dram_tensor("poolh", [128, NT * 8], BF, kind="ExternalInput")
    wd, r1bd = {}, {}
    for l in range(1, 6):
        din, dout = DIMS[l - 1]
        wd[l] = nc.dram_tensor(f"w{l}", [din, dout], BF, kind="ExternalInput")
        r1bd[l] = nc.dram_tensor(f"r1b{l}", [2, dout], BF, kind="ExternalInput")
    fw1 = nc.dram_tensor("fw1", [512, 512], FP, kind="ExternalInput")
    fb1 = nc.dram_tensor("fb1", [1, 512], FP, kind="ExternalInput")
    fw2 = nc.dram_tensor("fw2", [512, 1], FP, kind="ExternalInput")
    fb2 = nc.dram_tensor("fb2", [1, 1], FP, kind="ExternalInput")
    out_d = nc.dram_tensor("out", [8, 1], FP, kind="ExternalOutput")

    tabs = [nc.dram_tensor(f"t{l}", [NPT, widths[l - 1]], BF) for l in range(1, 6)]
    h5_d = nc.dram_tensor("h5", [NPT, 512], BF) if cfg.write_tables else None
    # a2a halves: flat layout = 8 chunks x [128, TQh, w] partition-major blocks
    a_inA = [nc.dram_tensor(f"a2ainA{l}", [C, 128, TQA, widths[l - 1]], BF) for l in range(1, 6)]
    a_inB = [nc.dram_tensor(f"a2ainB{l}", [C, 128, TQB, widths[l - 1]], BF) for l in range(1, 6)]
    a_outA = [nc.dram_tensor(f"a2aoutA{l}", [C, 128, TQA, widths[l - 1]], BF) for l in range(1, 6)]
    a_outB = [nc.dram_tensor(f"a2aoutB{l}", [C, 128, TQB, widths[l - 1]], BF) for l in range(1, 6)]

    with tile.TileContext(nc) as tc:
        with (
            tc.tile_pool(name="const", bufs=1) as const,
            tc.tile_pool(name="gat", bufs=2) as gpool,
            tc.tile_pool(name="sca", bufs=2) as spool,
            tc.tile_pool(name="zt", bufs=2) as zpool,
            tc.tile_pool(name="epi", bufs=4) as epool,
            tc.tile_pool(name="oute", bufs=4) as opool,
            tc.tile_pool(name="ps", bufs=6, space="PSUM") as ppool,
            tc.tile_pool(name="ps2", bufs=1, space="PSUM") as ppool2,
        ):
            gidx_sb = const.tile([128, NIW], I16)
            sidx_sb = const.tile([128, NIW], I16)
            disr_sb = const.tile([128, NT], FP)
            r1a_sb = const.tile([2, NPT], BF)
            poolh_sb = const.tile([128, NT, 8], BF)
            nc.sync.dma_start(gidx_sb[:], gidx[:, :])
            nc.sync.dma_start(sidx_sb[:], sidx[:, :])
            nc.sync.dma_start(disr_sb[:], disr[:, :])
            nc.sync.dma_start(r1a_sb[:], r1a[:, :])
            nc.sync.dma_start(poolh_sb[:], poolh[:, :].rearrange("p (t e) -> p t e", e=8))
            w_sb, r1b_sb = {}, {}
            for l in range(1, 6):
                din, dout = DIMS[l - 1]
                w_sb[l] = const.tile([128, din // 128, dout], BF, tag=f"w{l}", name=f"wsb{l}")
                nc.sync.dma_start(w_sb[l][:], wd[l][:, :].rearrange("(k p) n -> p k n", p=128))
                r1b_sb[l] = const.tile([2, dout], BF, tag=f"r1b{l}", name=f"r1bsb{l}")
                nc.sync.dma_start(r1b_sb[l][:], r1bd[l][:, :])
            fw1_sb = const.tile([128, 4, 512], FP)
            nc.sync.dma_start(fw1_sb[:], fw1[:, :].rearrange("(k p) n -> p k n", p=128))
            fw2_sb = const.tile([128, 4, 1], FP)
            nc.sync.dma_start(fw2_sb[:], fw2[:, :].rearrange("(k p) n -> p k n", p=128))
            fb1_sb = const.tile([1, 512], FP)
            nc.sync.dma_start(fb1_sb[:], fb1[:, :])
            fb2_sb = const.tile([1, 1], FP)
            nc.sync.dma_start(fb2_sb[:], fb2[:, :])
            ident_sb = const.tile([128, 128], FP)
            make_identity(nc, ident_sb[:])
            ones8_sb = const.tile([1, 8], FP)
            nc.vector.memset(ones8_sb[:], 1.0)

            nc.sync.dma_start(tabs[0][:, :], xt[:, :])
            zero_sb = const.tile([128, 512], BF)
            nc.vector.memset(zero_sb[:], 0.0)
            for li in range(1, 5):
                nc.sync.dma_start(tabs[li][cfg.np_cap : NPT, :], zero_sb[:, : widths[li]])

            pooled_ps = None
            for li in range(5):
                l = li + 1
                T = tabs[li]
                w = widths[li]
                din, dout = DIMS[li]
                K = din // 128

                # ---- Phase A: grouped gathers -> 2 AllToAlls -> waves ----
                # gather idx flat order: o*pair_cap + t*128 + p.  Small
                # instructions so Q7 desc-gen pipelines with ring drain.
                def emit_gather(o, t0, t1, half):
                    n = (t1 - t0) * 128
                    base = o * cfg.pair_cap + t0 * 128
                    g_t = gpool.tile([128, t1 - t0, w], BF, tag="gt", name="g_t")
                    nc.gpsimd.dma_gather(
                        out_ap=g_t[:],
                        in_ap=T[:, :],
                        idxs_ap=gidx_sb[:, base // 16 : (base + n) // 16],
                        num_idxs=n,
                        num_idxs_reg=n,
                        elem_size=w,
                        single_packet=False,
                    )
                    if half == 0:
                        nc.sync.dma_start(a_inA[li][o, :, t0:t1, :], g_t[:])
                    else:
                        nc.sync.dma_start(a_inB[li][o, :, t0 - TQA : t1 - TQA, :], g_t[:])

                gmid_a = TQA // 2
                gmid_b = TQA + (TQ - TQA) // 2
                for o in range(C):
                    emit_gather(o, 0, gmid_a, 0)
                    emit_gather(o, gmid_a, TQA, 0)
                nc.gpsimd.collective_compute(
                    "AllToAll",
                    mybir.AluOpType.bypass,
                    replica_groups=[list(range(C))],
                    ins=[a_inA[li].ap().opt()],
                    outs=[a_outA[li].ap().opt()],
                )
                for o in range(C):
                    emit_gather(o, TQA, gmid_b, 1)
                    emit_gather(o, gmid_b, TQ, 1)
                nc.gpsimd.collective_compute(
                    "AllToAll",
                    mybir.AluOpType.bypass,
                    replica_groups=[list(range(C))],
                    ins=[a_inB[li].ap().opt()],
                    outs=[a_outB[li].ap().opt()],
                )

                # scatter waves: t-range (a, b), two instructions per wave
                # (chunks 0-3 / 4-7) so gen overlaps drain
                spos = 0
                for wv, (a, b) in enumerate(tr):
                    ts_ = b - a
                    s_t = spool.tile([128, C, ts_, w], BF, tag="st", name="s_t")
                    if b <= TQA:
                        srcv = a_outA[li][:, :, a:b, :]
                    elif a >= TQA:
                        srcv = a_outB[li][:, :, a - TQA : b - TQA, :]
                    else:
                        raise AssertionError("wave straddles halves")
                    # per-chunk 3-level DMAs (a single 4-level AP hung on HW)
                    for o in range(C):
                        nc.sync.dma_start(s_t[:, o, :, :], srcv[o, :, :, :])
                    for og in range(2):
                        n = ts_ * 128 * (C // 2)
                        nc.gpsimd.dma_scatter_add(
                            out_ap=T[:, :],
                            in_ap=s_t[:, og * (C // 2) : (og + 1) * (C // 2), :, :].rearrange(
                                "p o t e -> p (o t) e"
                            ),
                            idxs_ap=sidx_sb[:, spos // 16 : (spos + n) // 16],
                            num_idxs=n,
                            num_idxs_reg=n,
                            elem_size=w,
                            single_packet=False,
                        )
                        spos += n

                # ---- Phase B: z^T load (quarters, pipelined), matmul ----
                if l == 5:
                    pooled_ps = ppool2.tile([128, 512], mybir.dt.float32, space="PSUM")
                NQ = min(4, NT)
                NTH = NT // NQ
                zT_parts = []
                for h in range(NQ):
                    r0 = h * NTH * 128
                    r1 = (h + 1) * NTH * 128 if h < NQ - 1 else cfg.np_cap
                    nth = (r1 - r0) // 128
                    zT = zpool.tile([128, K, nth * 128], BF, tag="zt", name="zT")
                    for kc in range(K):
                        nc.sync.dma_start_transpose(
                            zT[:, kc, :], T[r0:r1, kc * 128 : (kc + 1) * 128]
                        )
                    zT_parts.append((zT, r0 // 128, nth))
                for rt in range(NT):
                    zT, rt_base, _nth = zT_parts[min(rt // NTH, NQ - 1)]
                    rloc = rt - rt_base
                    ps = ppool.tile([128, dout], mybir.dt.float32, space="PSUM", tag="ps")
                    for kc in range(K):
                        nc.tensor.matmul(
                            ps[:],
                            lhsT=zT[:, kc, rloc * 128 : (rloc + 1) * 128],
                            rhs=w_sb[l][:, kc, :],
                            start=(kc == 0),
                            stop=False,
                        )
                    nc.tensor.matmul(
                        ps[:],
                        lhsT=r1a_sb[:, rt * 128 : (rt + 1) * 128],
                        rhs=r1b_sb[l][:],
                        start=False,
                        stop=True,
                    )
                    if l < 5:
                        e_t = epool.tile([128, dout], FP, tag="et")
                        nc.scalar.activation(
                            e_t[:], ps[:], AF.Relu, scale=disr_sb[:, rt : rt + 1]
                        )
                        o_t = opool.tile([128, dout], BF, tag="ot")
                        nc.vector.tensor_scalar_mul(o_t[:], e_t[:], disr_sb[:, rt : rt + 1])
                        nc.sync.dma_start(tabs[li + 1][rt * 128 : (rt + 1) * 128, :], o_t[:])
                    else:
                        o_t = opool.tile([128, dout], BF, tag="ot")
                        nc.scalar.activation(
                            o_t[:], ps[:], AF.Copy, scale=disr_sb[:, rt : rt + 1]
                        )
                        if cfg.write_tables:
                            nc.sync.dma_start(h5_d[rt * 128 : (rt + 1) * 128, :], o_t[:])
                        nc.tensor.matmul(
                            pooled_ps[:8, :],
                            lhsT=poolh_sb[:, rt, :],
                            rhs=o_t[:],
                            start=(rt == 0),
                            stop=(rt == NT - 1),
                        )

            # ---- head: relu -> fc1 -> relu -> fc2 ----
            pool_r = epool.tile([128, 512], FP, tag="pr")
            nc.scalar.activation(pool_r[:8, :], pooled_ps[:8, :], AF.Relu)
            pT = epool.tile([128, 4, 8], FP, tag="pT")
            for fc in range(4):
                tp = ppool.tile([128, 512], mybir.dt.float32, space="PSUM", tag="ps")
                nc.tensor.transpose(tp[:, :8], pool_r[:8, fc * 128 : (fc + 1) * 128], ident_sb[:8, :8])
                nc.vector.tensor_copy(pT[:, fc, :], tp[:, :8])
            hid_ps = ppool.tile([128, 512], mybir.dt.float32, space="PSUM", tag="ps")
            for fc in range(4):
                nc.tensor.matmul(
                    hid_ps[:8, :], lhsT=pT[:, fc, :], rhs=fw1_sb[:, fc, :],
                    start=(fc == 0), stop=False,
                )
            nc.tensor.matmul(hid_ps[:8, :], lhsT=ones8_sb[:1, :], rhs=fb1_sb[:1, :], start=False, stop=True)
            hid_r = epool.tile([128, 512], FP, tag="pr")
            nc.scalar.activation(hid_r[:8, :], hid_ps[:8, :], AF.Relu)
            hT = epool.tile([128, 4, 8], FP, tag="pT")
            for fc in range(4):
                tp = ppool.tile([128, 512], mybir.dt.float32, space="PSUM", tag="ps")
                nc.tensor.transpose(tp[:, :8], hid_r[:8, fc * 128 : (fc + 1) * 128], ident_sb[:8, :8])
                nc.vector.tensor_copy(hT[:, fc, :], tp[:, :8])
            out_ps = ppool.tile([128, 512], mybir.dt.float32, space="PSUM", tag="ps")
            for fc in range(4):
                nc.tensor.matmul(
                    out_ps[:8, :1], lhsT=hT[:, fc, :], rhs=fw2_sb[:, fc, :],
                    start=(fc == 0), stop=False,
                )
            nc.tensor.matmul(out_ps[:8, :1], lhsT=ones8_sb[:1, :], rhs=fb2_sb[:1, :], start=False, stop=True)
            out_sb = epool.tile([128, 1], FP, tag="osb")
            nc.vector.tensor_copy(out_sb[:8, :], out_ps[:8, :1])
            nc.sync.dma_start(out_d[:, :], out_sb[:8, :])

    nc.compile()
    return nc


_NC_CACHE = {}


def _get_nc(cfg: Cfg):
    if cfg not in _NC_CACHE:
        _NC_CACHE[cfg] = build(cfg)
    return _NC_CACHE[cfg]


def kernel(x, ei_intra, ei_inter, batch, params):
    cfg = FULL_CFG
    in_maps = preprocess(x, ei_intra, ei_inter, batch, params, cfg)
    nc = _get_nc(cfg)
    from concourse.bass_utils import run_bass_kernel_spmd

    res = run_bass_kernel_spmd(nc, in_maps, core_ids=list(range(cfg.c)))
    outs = [np.asarray(res.results[c]["out"], np.float32).reshape(8) for c in range(cfg.c)]
    return np.concatenate(outs)
